# revision 17
# baseline (speedup 1.0000x reference)
"""Single-head causal attention (prefill) on 8 Trainium2 NeuronCores.

Problem: x[4,4096,2048], Wq/Wk/Wv[2048,128] -> out[4,4096,128]
  Q=xWq K=xWk V=xWv; out = softmax(mask(QK^T/sqrt(128))) V

Sharding: data-parallel over batch (4) x 2-way balanced causal query split.
Core c handles batch c//2; half h=c%2 takes query chunks (of 512)
  h=0: {0,3,4,7}   h=1: {1,2,5,6}
which balances causal attention FLOPs. A single SPMD program serves both
halves: slot s processes 512 queries against the first NK[s] key columns
(NK=[1024,2048,3072,4096]); the host permutes the key columns of its
pre-transposed x (h=1 swaps adjacent 512-blocks) so each core's needed keys
always form a prefix, and per-core causal masks are threshold DATA
(thr[s,i,k]: query column threshold per key row) applied on-device as
  e = (iota >= thr) * exp(scores)

All PE-streamed operands are fp16 (1 cycle/column on the PE vs ~2 for
fp32r; fp16 keeps 11 mantissa bits so 0..2048 integers such as the mask
iota/thresholds stay exact). PSUM accumulation is fp32 throughout.

Scheduling: the PE engine queue is strict FIFO, so attention chunks are
woven between projection q4-steps at emission time (proj group order
[0,3,1,2,4,7,5,6] makes every slot's QT available early); each chunk's
AV matmul is emitted ~1 weave step after its scores matmul so the PE
never stalls waiting on the ACT exp / DVE mask of the same chunk.
"""

import numpy as np

B, T, E, D = 4, 4096, 2048, 128
NCORES = 8
G = 512                     # query-group / t-group width
NG = T // G                 # 8 t-groups
EC = E // 128               # 16 e-chunks
QSLOTS = [0, 3, 4, 7]       # t-group holding slot s's queries (canonical order)
NK = [1024, 2048, 3072, 4096]  # key-prefix length per slot
NMASK = 8                   # mask applied to the last 8 k-chunks of each slot
CHUNKS = [[0, 3, 4, 7], [1, 2, 5, 6]]    # global query chunk of (h, slot)
PERMS = [[0, 1, 2, 3, 4, 5, 6, 7], [1, 0, 3, 2, 5, 4, 7, 6]]  # key col perm per h
SCALE = 1.0 / float(np.sqrt(D))

PO = [0, 3, 1, 2, 4, 7, 5, 6]       # projection group emission order
DOSE = [0, 1, 2, 3, 3, 4, 6, 6]     # attn chunks woven per q4-step, by PO index
TAILN = 12                          # last slot-3 chunks use PE-matmul denominator

_CACHE = {}


def _schedule():
    """Linear emission program: ('p',g,q4) proj steps with ('a',s,c) attention
    chunks woven in as soon as their KT/QT group dependencies are emitted.
    Chunks within a slot may be emitted out of order (PSUM accumulation and
    the denominator chain are order-insensitive)."""
    nch = [n // 128 for n in NK]
    done = set()
    left = [list(range(n)) for n in nch]
    ops = []

    def drain(budget):
        n = 0
        while n < budget:
            for s in range(4):
                if QSLOTS[s] not in done:
                    continue
                c = next((c for c in left[s] if (c // 4) in done), None)
                if c is not None:
                    left[s].remove(c)
                    ops.append(("a", s, c))
                    n += 1
                    break
            else:
                return
    for idx, g in enumerate(PO):
        for q4 in range(4):
            ops.append(("p", g, q4))
            drain(DOSE[idx])
        done.add(g)
    drain(1 << 30)
    assert all(not l for l in left)
    # mark the last TAILN slot-3 chunks: their denominator contribution goes
    # through an accumulating PE ones-matmul (the PE idles in the tail) rather
    # than the DVE chain, which gates the tail cadence otherwise
    mmden = set()
    for i in range(len(ops) - 1, -1, -1):
        if ops[i][0] == "a" and ops[i][1] == 3:
            mmden.add((3, ops[i][2]))
            if len(mmden) == TAILN:
                break
    return ops, mmden


def _emit(nc, tc, ctx, aps):
    import concourse.bass as bass  # noqa: F401
    from concourse import mybir

    f32 = mybir.dt.float32
    f16 = mybir.dt.float16
    xt, wq, wk, wv, msk, ident, out = (
        aps["xt"], aps["wq"], aps["wk"], aps["wv"], aps["msk"], aps["ident"],
        aps["out"],
    )

    # ---- pools ----
    wpool = ctx.enter_context(tc.tile_pool(name="w", bufs=1))
    cpool = ctx.enter_context(tc.tile_pool(name="const", bufs=1))
    xpool = ctx.enter_context(tc.tile_pool(name="xt", bufs=8))
    ktpool = ctx.enter_context(tc.tile_pool(name="kt", bufs=NG))
    vtpool = ctx.enter_context(tc.tile_pool(name="vt", bufs=2))
    vnpool = ctx.enter_context(tc.tile_pool(name="vn", bufs=NG))
    qtpool = ctx.enter_context(tc.tile_pool(name="qt", bufs=4))
    epool = ctx.enter_context(tc.tile_pool(name="e", bufs=6))
    espool = ctx.enter_context(tc.tile_pool(name="es", bufs=4))
    dpool_sb = ctx.enter_context(tc.tile_pool(name="dsb", bufs=2))
    opool_sb = ctx.enter_context(tc.tile_pool(name="osb", bufs=2))

    ppool = ctx.enter_context(tc.tile_pool(name="pp", bufs=3, space="PSUM"))
    spool = ctx.enter_context(tc.tile_pool(name="sp", bufs=2, space="PSUM"))
    apool = ctx.enter_context(tc.tile_pool(name="av", bufs=2, space="PSUM"))
    denp = ctx.enter_context(tc.tile_pool(name="den", bufs=1, space="PSUM"))

    # ---- constants ----
    w_sb = {}
    for name, ap in (("wq", wq), ("wk", wk), ("wv", wv)):
        t = wpool.tile([128, EC, 128], f16, tag=name, name=name)
        w_sb[name] = t
    for r in range(0, EC, 4):
        for name, ap in (("wk", wk), ("wv", wv), ("wq", wq)):
            nc.gpsimd.dma_start(
                out=w_sb[name][:, r:r + 4, :], in_=ap[:, r:r + 4, :])
    identity = cpool.tile([128, 128], f16, tag="ident", name="ident")
    nc.gpsimd.dma_start(out=identity[:, :], in_=ident[:, :])
    msk_sb = cpool.tile([128, 4, NMASK, G], f16, tag="msk", name="msk")
    nc.gpsimd.dma_start(out=msk_sb[:, :, :, :], in_=msk[:, :, :, :])
    ones_f = cpool.tile([128, 1], f32, tag="ones_f", name="ones_f")
    nc.vector.memset(ones_f[:, :], 1.0)
    ones = cpool.tile([128, 1], f16, tag="ones", name="ones")
    nc.vector.tensor_copy(out=ones[:, :], in_=ones_f[:, :])
    # Warm the PE HAM clock gate (~3.4us of busy flips K from 4/8 to 8/8)
    # with dummy matmuls while the first x/W DMAs are still streaming.
    wsc = cpool.tile([128, G], f16, tag="wsc", name="wsc")
    nc.vector.memset(wsc[:, :], 1.0)
    wps = spool.tile([128, G], f32, tag="sp", name="wps")
    for _ in range(14):
        nc.tensor.matmul(
            wps[:, :], wsc[:, 0:128], wsc[:, :], start=True, stop=True)

    KT = [None] * NG   # [d=128, G] per t-group
    VN = [None] * NG   # V natural [t_in=128, 4*128 d-cols]
    QT = [None] * 4    # [d=128, G] per slot

    def proj_step(g, q4):
        is_q = g in QSLOTS
        if q4 == 0:
            st["pk"] = ppool.tile([128, G], f32, tag="pp", name="pp")
            st["pv"] = ppool.tile([128, G], f32, tag="pp", name="pp")
            st["pq"] = (
                ppool.tile([128, G], f32, tag="pp", name="pp") if is_q else None)
        pk, pv, pq = st["pk"], st["pv"], st["pq"]
        xtile = xpool.tile([128, 4, G], f16, tag="xt", name="xt")
        if st.get("nsteps", 0) < 2:
            # split the first steps' loads so the first matmul can start
            # after 1/4 of the tile has landed
            for i in range(4):
                nc.sync.dma_start(
                    out=xtile[:, i:i + 1, :], in_=xt[:, g, q4, i:i + 1, :])
        else:
            nc.sync.dma_start(out=xtile[:, :, :], in_=xt[:, g, q4, :, :])
        st["nsteps"] = st.get("nsteps", 0) + 1
        for i in range(4):
            j = q4 * 4 + i
            rhs = xtile[:, i, :]
            stt, spp = j == 0, j == EC - 1
            nc.tensor.matmul(
                pk[:, :], w_sb["wk"][:, j, :], rhs, start=stt, stop=spp)
            nc.tensor.matmul(
                pv[:, :], w_sb["wv"][:, j, :], rhs, start=stt, stop=spp)
            if is_q:
                nc.tensor.matmul(
                    pq[:, :], w_sb["wq"][:, j, :], rhs, start=stt, stop=spp)
        if q4 < 3:
            return
        # group complete: copy projections out of PSUM, transpose V
        kt = ktpool.tile([128, G], f16, tag="kt", name="kt")
        nc.scalar.copy(out=kt[:, :], in_=pk[:, :])
        KT[g] = kt
        vt = vtpool.tile([128, G], f16, tag="vt", name="vt")
        nc.scalar.copy(out=vt[:, :], in_=pv[:, :])
        if is_q:
            qt = qtpool.tile([128, G], f16, tag="qt", name="qt")
            nc.scalar.copy(out=qt[:, :], in_=pq[:, :])
            QT[QSLOTS.index(g)] = qt
        vn = vnpool.tile([128, G], f16, tag="vn", name="vn")
        for c in range(4):
            pt = spool.tile([128, 128], f16, tag="sp", name="tp")
            nc.tensor.transpose(
                pt[:, :], vt[:, c * 128:(c + 1) * 128], identity[:, :])
            nc.vector.tensor_copy(
                out=vn[:, c * 128:(c + 1) * 128], in_=pt[:, :])
        VN[g] = vn

    slot_st = [dict(po=None, acc=None, pd3=None, nsc=0, nav=0)
               for _ in range(4)]

    def attn_sc(s, c, mm):
        """Scores matmul + exp + mask + denominator chain for chunk c."""
        nchunks = NK[s] // 128
        g, cc = c // 4, c % 4
        ps = spool.tile([128, G], f32, tag="sp", name="sp")
        nc.tensor.matmul(
            ps[:, :], KT[g][:, cc * 128:(cc + 1) * 128], QT[s][:, :],
            start=True, stop=True)
        e = epool.tile([128, G], f16, tag="e", name="e")
        nc.scalar.activation(
            out=e[:, :], in_=ps[:, :],
            func=mybir.ActivationFunctionType.Exp, scale=SCALE)
        mi = c - (nchunks - NMASK)
        if mi >= 0:
            nc.vector.tensor_mul(
                e[:, :], e[:, :], msk_sb[:, s, mi, :])
        if mm:
            return e   # denominator via PE ones-matmul at AV flush time
        if slot_st[s]["nsc"] == 0:
            slot_st[s]["acc"] = e
        else:
            esum = espool.tile([128, G], f16, tag="es", name="es")
            nc.vector.tensor_add(esum[:, :], slot_st[s]["acc"][:, :], e[:, :])
            slot_st[s]["acc"] = esum
        slot_st[s]["nsc"] += 1
        return e

    def attn_av(s, c, e, mm):
        nchunks = NK[s] // 128
        g, cc = c // 4, c % 4
        n = slot_st[s]["nav"]
        if n == 0:
            slot_st[s]["po"] = apool.tile([128, G], f32, tag="av", name="av")
        nc.tensor.matmul(
            slot_st[s]["po"][:, :], VN[g][:, cc * 128:(cc + 1) * 128], e[:, :],
            start=(n == 0), stop=(n == nchunks - 1))
        slot_st[s]["nav"] = n + 1
        if mm:
            first = slot_st[s]["pd3"] is None
            if first:
                slot_st[s]["pd3"] = ppool.tile([1, G], f32, tag="pp", name="pd3")
            nc.tensor.matmul(
                slot_st[s]["pd3"][:, :], ones[:, :], e[:, :],
                start=first, stop=False)

    def attn_fin(s):
        if slot_st[s]["pd3"] is not None:
            pd = slot_st[s]["pd3"]
            nc.tensor.matmul(
                pd[:, :], ones[:, :], slot_st[s]["acc"][:, :],
                start=False, stop=True)
        else:
            pd = denp.tile([1, G], f32, tag="den", name="den")
            nc.tensor.matmul(
                pd[:, :], ones[:, :], slot_st[s]["acc"][:, :],
                start=True, stop=True)
        dr = dpool_sb.tile([1, G], f32, tag="dr", name="dr")
        nc.vector.tensor_copy(out=dr[:, :], in_=pd[:, :])
        rr = dpool_sb.tile([1, G], f32, tag="rr", name="rr")
        rs = dpool_sb.tile([1, G], f32, tag="rs", name="rs")
        nc.vector.reciprocal_approx_accurate(
            out=rr[:, :], in_=dr[:, :], scratch=rs[:, :])
        db = dpool_sb.tile([128, G], f32, tag="db", name="db")
        nc.gpsimd.partition_broadcast(db[:, :], rr[:, :])
        osb = opool_sb.tile([128, G], f32, tag="osb", name="osb")
        nc.vector.tensor_mul(osb[:, :], slot_st[s]["po"][:, :], db[:, :])
        nc.sync.dma_start(out=out[s, :, :], in_=osb[:, :])

    # ---- emit the woven program with AV skewed one op behind SC ----
    st = {}
    pend = []          # [(s, c, e, mm), ...] chunks whose AV is not yet emitted

    def flush_av(keep):
        while len(pend) > keep:
            s, c, e, mm = pend.pop(0)
            attn_av(s, c, e, mm)
            if slot_st[s]["nav"] == NK[s] // 128:
                attn_fin(s)

    ops, mmden = _schedule()
    for op in ops:
        if op[0] == "p":
            flush_av(1)
            proj_step(op[1], op[2])
        else:
            s, c = op[1], op[2]
            flush_av(1)
            mm = (s, c) in mmden
            e = attn_sc(s, c, mm)
            pend.append((s, c, e, mm))
    flush_av(0)


def _build():
    if "nc" in _CACHE:
        return _CACHE["nc"]
    from contextlib import ExitStack

    import concourse.bacc as bacc
    import concourse.tile as tile
    from concourse import mybir

    f32 = mybir.dt.float32
    f16 = mybir.dt.float16
    nc = bacc.Bacc(
        "TRN2", target_bir_lowering=False, debug=False, enable_asserts=False,
        num_devices=NCORES,
    )
    aps = {
        "xt": nc.dram_tensor(
            "xt", [128, NG, 4, 4, G], f16, kind="ExternalInput").ap(),
        "wq": nc.dram_tensor("wq", [128, EC, D], f16, kind="ExternalInput").ap(),
        "wk": nc.dram_tensor("wk", [128, EC, D], f16, kind="ExternalInput").ap(),
        "wv": nc.dram_tensor("wv", [128, EC, D], f16, kind="ExternalInput").ap(),
        "msk": nc.dram_tensor(
            "msk", [128, 4, NMASK, G], f16, kind="ExternalInput").ap(),
        "ident": nc.dram_tensor(
            "ident", [128, 128], f16, kind="ExternalInput").ap(),
        "out": nc.dram_tensor("out", [4, 128, G], f32, kind="ExternalOutput").ap(),
    }
    with tile.TileContext(nc) as tc, ExitStack() as ctx:
        _emit(nc, tc, ctx, aps)
    nc.compile()
    _CACHE["nc"] = nc
    return nc


def _thresholds(h):
    """thr[s, i, k_in]: min allowed local query col for key row k_in of the
    i-th masked k-chunk (chunk c = NK[s]/128 - NMASK + i) of slot s."""
    perm = np.asarray(PERMS[h])
    thr = np.zeros((4, NMASK, 128), dtype=np.float32)
    for s in range(4):
        qc = CHUNKS[h][s]
        c0 = NK[s] // 128 - NMASK
        for i in range(NMASK):
            pos = (c0 + i) * 128 + np.arange(128)        # permuted key column
            k_orig = perm[pos // G] * G + pos % G        # original key index
            thr[s, i] = np.clip(k_orig - qc * G, 0, G)
    return thr


def make_in_maps(x, Wq, Wk, Wv):
    x = np.ascontiguousarray(x, dtype=np.float32)
    def wshape(W):
        # [E, D] -> [128, EC, D]: chunk c rows c*128..c*128+127 at [:, c, :]
        return np.ascontiguousarray(
            np.asarray(W, dtype=np.float32).reshape(EC, 128, D)
            .transpose(1, 0, 2).astype(np.float16))

    common = {
        "wq": wshape(Wq), "wk": wshape(Wk), "wv": wshape(Wv),
        "ident": np.eye(128, dtype=np.float16),
    }
    # msk[k, s, i, q] = 1.0 where local query col q is allowed for key row k
    # of masked chunk i of slot s  (== iota_q >= thr[s, i, k])
    msks = []
    for h in range(2):
        thr = _thresholds(h)                              # [4, NMASK, 128]
        m = (np.arange(G)[None, None, None, :]
             >= thr[:, :, :, None]).astype(np.float16)    # [4, NMASK, 128, G]
        msks.append(np.ascontiguousarray(m.transpose(2, 0, 1, 3)))
    in_maps = []
    for c in range(NCORES):
        b, h = c // 2, c % 2
        xr = x[b].reshape(NG, G, E)[PERMS[h]]            # [g, t, E] permuted
        # xt[p, g, q4, i, t] = xr[g, t, (q4*4+i)*128+p]: each partition's
        # (g, q4) line is 4*512*2B = 4KB contiguous for efficient DMA.
        xt = np.ascontiguousarray(
            xr.astype(np.float16).reshape(NG, G, 4, 4, 128)
            .transpose(4, 0, 2, 3, 1))
        in_maps.append({**common, "xt": xt, "msk": msks[h]})
    return in_maps


def gather(results):
    out = np.empty((B, T, D), dtype=np.float32)
    for c in range(NCORES):
        b, h = c // 2, c % 2
        o = results[c]["out"]                             # [4, 128, 512]
        for s in range(4):
            qc = CHUNKS[h][s]
            out[b, qc * G:(qc + 1) * G, :] = o[s].T
    return out


def run(x, Wq, Wk, Wv, trace=False, **trace_kwargs):
    from concourse.bass_utils import run_bass_kernel_spmd

    nc = _build()
    in_maps = make_in_maps(x, Wq, Wk, Wv)
    res = run_bass_kernel_spmd(
        nc, in_maps, core_ids=list(range(NCORES)), trace=trace, **trace_kwargs)
    return gather(res.results), res


def kernel(x, Wq, Wk, Wv):
    out, _ = run(np.asarray(x), np.asarray(Wq), np.asarray(Wk), np.asarray(Wv))
    return out


# revision 24
# speedup vs baseline: 1.0695x; 1.0695x over previous
"""Single-head causal attention (prefill) on 8 Trainium2 NeuronCores.

Problem: x[4,4096,2048], Wq/Wk/Wv[2048,128] -> out[4,4096,128]
  Q=xWq K=xWk V=xWv; out = softmax(mask(QK^T/sqrt(128))) V

Sharding: data-parallel over batch (4) x 2-way balanced causal query split.
Core c handles batch c//2; half h=c%2 takes query chunks (of 512)
  h=0: {0,3,4,7}   h=1: {1,2,5,6}
which balances causal attention FLOPs. A single SPMD program serves both
halves: slot s processes 512 queries against the first NK[s] key columns
(NK=[1024,2048,3072,4096]); the host permutes the key columns of its
pre-transposed x (h=1 swaps adjacent 512-blocks) so each core's needed keys
always form a prefix, and per-core causal masks are threshold DATA
(thr[s,i,k]: query column threshold per key row) applied on-device as
  e = (iota >= thr) * exp(scores)

All PE-streamed operands are fp16 (1 cycle/column on the PE vs ~2 for
fp32r; fp16 keeps 11 mantissa bits so 0..2048 integers such as the mask
iota/thresholds stay exact). PSUM accumulation is fp32 throughout.

Scheduling: the PE engine queue is strict FIFO, so attention chunks are
woven between projection q4-steps at emission time (proj group order
[0,3,1,2,4,7,5,6] makes every slot's QT available early); each chunk's
AV matmul is emitted ~1 weave step after its scores matmul so the PE
never stalls waiting on the ACT exp / DVE mask of the same chunk.
"""

import numpy as np

B, T, E, D = 4, 4096, 2048, 128
NCORES = 8
G = 512                     # query-group / t-group width
NG = T // G                 # 8 t-groups
EC = E // 128               # 16 e-chunks
QSLOTS = [0, 3, 4, 7]       # t-group holding slot s's queries (canonical order)
NK = [1024, 2048, 3072, 4096]  # key-prefix length per slot
NMASK = 8                   # mask applied to the last 8 k-chunks of each slot
CHUNKS = [[0, 3, 4, 7], [1, 2, 5, 6]]    # global query chunk of (h, slot)
PERMS = [[0, 1, 2, 3, 4, 5, 6, 7], [1, 0, 3, 2, 5, 4, 7, 6]]  # key col perm per h
SCALE = 1.0 / float(np.sqrt(D))

PO = [0, 3, 1, 2, 4, 7, 5, 6]       # projection group emission order
DOSE = [0, 1, 2, 3, 3, 4, 6, 6]     # attn chunks woven per q4-step, by PO index
TAILN = 12                          # last slot-3 chunks use PE-matmul denominator

_CACHE = {}


def _schedule():
    """Linear emission program: ('p',g,q4) proj steps with ('a',s,c) attention
    chunks woven in as soon as their KT/QT group dependencies are emitted.
    Chunks within a slot may be emitted out of order (PSUM accumulation and
    the denominator chain are order-insensitive)."""
    nch = [n // 128 for n in NK]
    done = set()
    left = [list(range(n)) for n in nch]
    ops = []

    def drain(budget):
        n = 0
        while n < budget:
            for s in range(4):
                if QSLOTS[s] not in done:
                    continue
                c = next((c for c in left[s] if (c // 4) in done), None)
                if c is not None:
                    left[s].remove(c)
                    ops.append(("a", s, c))
                    n += 1
                    break
            else:
                return
    for idx, g in enumerate(PO):
        for q4 in range(4):
            ops.append(("p", g, q4))
            drain(DOSE[idx])
        done.add(g)
    drain(1 << 30)
    assert all(not l for l in left)
    # mark the last TAILN slot-3 chunks: their denominator contribution goes
    # through an accumulating PE ones-matmul (the PE idles in the tail) rather
    # than the DVE chain, which gates the tail cadence otherwise
    mmden = set()
    for i in range(len(ops) - 1, -1, -1):
        if ops[i][0] == "a" and ops[i][1] == 3:
            mmden.add((3, ops[i][2]))
            if len(mmden) == TAILN:
                break
    return ops, mmden


def _emit(nc, tc, ctx, aps):
    import concourse.bass as bass  # noqa: F401
    from concourse import mybir

    f32 = mybir.dt.float32
    f16 = mybir.dt.float16
    xt, wq, wk, wv, thr, ident, out = (
        aps["xt"], aps["wq"], aps["wk"], aps["wv"], aps["thr"], aps["ident"],
        aps["out"],
    )

    # ---- pools ----
    wpool = ctx.enter_context(tc.tile_pool(name="w", bufs=1))
    cpool = ctx.enter_context(tc.tile_pool(name="const", bufs=1))
    xpool = ctx.enter_context(tc.tile_pool(name="xt", bufs=8))
    ktpool = ctx.enter_context(tc.tile_pool(name="kt", bufs=NG))
    vtpool = ctx.enter_context(tc.tile_pool(name="vt", bufs=2))
    vnpool = ctx.enter_context(tc.tile_pool(name="vn", bufs=NG))
    qtpool = ctx.enter_context(tc.tile_pool(name="qt", bufs=4))
    epool = ctx.enter_context(tc.tile_pool(name="e", bufs=6))
    espool = ctx.enter_context(tc.tile_pool(name="es", bufs=4))
    dpool_sb = ctx.enter_context(tc.tile_pool(name="dsb", bufs=2))
    opool_sb = ctx.enter_context(tc.tile_pool(name="osb", bufs=2))

    ppool = ctx.enter_context(tc.tile_pool(name="pp", bufs=3, space="PSUM"))
    spool = ctx.enter_context(tc.tile_pool(name="sp", bufs=2, space="PSUM"))
    apool = ctx.enter_context(tc.tile_pool(name="av", bufs=2, space="PSUM"))
    denp = ctx.enter_context(tc.tile_pool(name="den", bufs=1, space="PSUM"))

    # ---- constants ----
    w_sb = {}
    for name, ap in (("wq", wq), ("wk", wk), ("wv", wv)):
        t = wpool.tile([128, EC, 128], f16, tag=name, name=name)
        w_sb[name] = t
    for r in range(0, EC, 4):
        for name, ap in (("wk", wk), ("wv", wv), ("wq", wq)):
            nc.gpsimd.dma_start(
                out=w_sb[name][:, r:r + 4, :], in_=ap[:, r:r + 4, :])
    identity = cpool.tile([128, 128], f16, tag="ident", name="ident")
    nc.gpsimd.dma_start(out=identity[:, :], in_=ident[:, :])
    thr_sb = cpool.tile([128, 4, NMASK], f16, tag="thr", name="thr")
    nc.gpsimd.dma_start(out=thr_sb[:, :, :], in_=thr.rearrange("s i k -> k s i"))
    iota_f = cpool.tile([128, G], f32, tag="iota_f", name="iota_f")
    nc.gpsimd.iota(
        iota_f[:, :], pattern=[[1, G]], base=0, channel_multiplier=0,
        allow_small_or_imprecise_dtypes=True,
    )
    iota = cpool.tile([128, G], f16, tag="iota", name="iota")
    nc.vector.tensor_copy(out=iota[:, :], in_=iota_f[:, :])
    ones_f = cpool.tile([128, 1], f32, tag="ones_f", name="ones_f")
    nc.vector.memset(ones_f[:, :], 1.0)
    ones = cpool.tile([128, 1], f16, tag="ones", name="ones")
    nc.vector.tensor_copy(out=ones[:, :], in_=ones_f[:, :])
    # Warm the PE HAM clock gate (~3.4us of busy flips K from 4/8 to 8/8)
    # with dummy matmuls while the first x/W DMAs are still streaming.
    wsc = cpool.tile([128, G], f16, tag="wsc", name="wsc")
    nc.vector.memset(wsc[:, :], 1.0)
    wps = spool.tile([128, G], f32, tag="sp", name="wps")
    for _ in range(14):
        nc.tensor.matmul(
            wps[:, :], wsc[:, 0:128], wsc[:, :], start=True, stop=True)

    KT = [None] * NG   # [d=128, G] per t-group
    VN = [None] * NG   # V natural [t_in=128, 4*128 d-cols]
    QT = [None] * 4    # [d=128, G] per slot

    def proj_step(g, q4):
        is_q = g in QSLOTS
        if q4 == 0:
            st["pk"] = ppool.tile([128, G], f32, tag="pp", name="pp")
            st["pv"] = ppool.tile([128, G], f32, tag="pp", name="pp")
            st["pq"] = (
                ppool.tile([128, G], f32, tag="pp", name="pp") if is_q else None)
        pk, pv, pq = st["pk"], st["pv"], st["pq"]
        xtile = xpool.tile([128, 4, G], f16, tag="xt", name="xt")
        nc.sync.dma_start(out=xtile[:, :, :], in_=xt[:, g, q4, :, :])
        for i in range(4):
            j = q4 * 4 + i
            rhs = xtile[:, i, :]
            stt, spp = j == 0, j == EC - 1
            nc.tensor.matmul(
                pk[:, :], w_sb["wk"][:, j, :], rhs, start=stt, stop=spp)
            nc.tensor.matmul(
                pv[:, :], w_sb["wv"][:, j, :], rhs, start=stt, stop=spp)
            if is_q:
                nc.tensor.matmul(
                    pq[:, :], w_sb["wq"][:, j, :], rhs, start=stt, stop=spp)
        if q4 < 3:
            return
        # group complete: copy projections out of PSUM, transpose V
        kt = ktpool.tile([128, G], f16, tag="kt", name="kt")
        nc.scalar.copy(out=kt[:, :], in_=pk[:, :])
        KT[g] = kt
        vt = vtpool.tile([128, G], f16, tag="vt", name="vt")
        nc.scalar.copy(out=vt[:, :], in_=pv[:, :])
        if is_q:
            qt = qtpool.tile([128, G], f16, tag="qt", name="qt")
            nc.scalar.copy(out=qt[:, :], in_=pq[:, :])
            QT[QSLOTS.index(g)] = qt
        vn = vnpool.tile([128, G], f16, tag="vn", name="vn")
        for c in range(4):
            pt = spool.tile([128, 128], f16, tag="sp", name="tp")
            nc.tensor.transpose(
                pt[:, :], vt[:, c * 128:(c + 1) * 128], identity[:, :])
            nc.vector.tensor_copy(
                out=vn[:, c * 128:(c + 1) * 128], in_=pt[:, :])
        VN[g] = vn

    slot_st = [dict(po=None, acc=None, pd3=None, nsc=0, nav=0)
               for _ in range(4)]

    def attn_sc(s, c, mm):
        """Scores matmul + exp + mask + denominator chain for chunk c."""
        nchunks = NK[s] // 128
        g, cc = c // 4, c % 4
        ps = spool.tile([128, G], f32, tag="sp", name="sp")
        nc.tensor.matmul(
            ps[:, :], KT[g][:, cc * 128:(cc + 1) * 128], QT[s][:, :],
            start=True, stop=True)
        e = epool.tile([128, G], f16, tag="e", name="e")
        nc.scalar.activation(
            out=e[:, :], in_=ps[:, :],
            func=mybir.ActivationFunctionType.Exp, scale=SCALE)
        mi = c - (nchunks - NMASK)
        if mi >= 0:
            nc.vector.scalar_tensor_tensor(
                out=e[:, :], in0=iota[:, :], scalar=thr_sb[:, s, mi:mi + 1],
                in1=e[:, :], op0=mybir.AluOpType.is_ge,
                op1=mybir.AluOpType.mult)
        if mm:
            return e   # denominator via PE ones-matmul at AV flush time
        if slot_st[s]["nsc"] == 0:
            slot_st[s]["acc"] = e
        else:
            esum = espool.tile([128, G], f16, tag="es", name="es")
            nc.vector.tensor_add(esum[:, :], slot_st[s]["acc"][:, :], e[:, :])
            slot_st[s]["acc"] = esum
        slot_st[s]["nsc"] += 1
        return e

    def attn_av(s, c, e, mm):
        nchunks = NK[s] // 128
        g, cc = c // 4, c % 4
        n = slot_st[s]["nav"]
        if n == 0:
            slot_st[s]["po"] = apool.tile([128, G], f32, tag="av", name="av")
        nc.tensor.matmul(
            slot_st[s]["po"][:, :], VN[g][:, cc * 128:(cc + 1) * 128], e[:, :],
            start=(n == 0), stop=(n == nchunks - 1))
        slot_st[s]["nav"] = n + 1
        if mm:
            first = slot_st[s]["pd3"] is None
            if first:
                slot_st[s]["pd3"] = ppool.tile([1, G], f32, tag="pp", name="pd3")
            nc.tensor.matmul(
                slot_st[s]["pd3"][:, :], ones[:, :], e[:, :],
                start=first, stop=False)

    def attn_fin(s):
        if slot_st[s]["pd3"] is not None:
            pd = slot_st[s]["pd3"]
            nc.tensor.matmul(
                pd[:, :], ones[:, :], slot_st[s]["acc"][:, :],
                start=False, stop=True)
        else:
            pd = denp.tile([1, G], f32, tag="den", name="den")
            nc.tensor.matmul(
                pd[:, :], ones[:, :], slot_st[s]["acc"][:, :],
                start=True, stop=True)
        dr = dpool_sb.tile([1, G], f32, tag="dr", name="dr")
        nc.vector.tensor_copy(out=dr[:, :], in_=pd[:, :])
        rr = dpool_sb.tile([1, G], f32, tag="rr", name="rr")
        rs = dpool_sb.tile([1, G], f32, tag="rs", name="rs")
        nc.vector.reciprocal_approx_accurate(
            out=rr[:, :], in_=dr[:, :], scratch=rs[:, :])
        db = dpool_sb.tile([128, G], f32, tag="db", name="db")
        nc.gpsimd.partition_broadcast(db[:, :], rr[:, :])
        osb = opool_sb.tile([128, G], f32, tag="osb", name="osb")
        nc.vector.tensor_mul(osb[:, :], slot_st[s]["po"][:, :], db[:, :])
        nc.sync.dma_start(out=out[s, :, :], in_=osb[:, :])

    # ---- emit the woven program with AV skewed one op behind SC ----
    st = {}
    pend = []          # [(s, c, e, mm), ...] chunks whose AV is not yet emitted

    def flush_av(keep):
        while len(pend) > keep:
            s, c, e, mm = pend.pop(0)
            attn_av(s, c, e, mm)
            if slot_st[s]["nav"] == NK[s] // 128:
                attn_fin(s)

    ops, mmden = _schedule()
    for op in ops:
        if op[0] == "p":
            flush_av(1)
            proj_step(op[1], op[2])
        else:
            s, c = op[1], op[2]
            flush_av(1)
            mm = (s, c) in mmden
            e = attn_sc(s, c, mm)
            pend.append((s, c, e, mm))
    flush_av(0)


def _build():
    if "nc" in _CACHE:
        return _CACHE["nc"]
    from contextlib import ExitStack

    import concourse.bacc as bacc
    import concourse.tile as tile
    from concourse import mybir

    f32 = mybir.dt.float32
    f16 = mybir.dt.float16
    nc = bacc.Bacc(
        "TRN2", target_bir_lowering=False, debug=False, enable_asserts=False,
        num_devices=NCORES,
    )
    aps = {
        "xt": nc.dram_tensor(
            "xt", [128, NG, 4, 4, G], f16, kind="ExternalInput").ap(),
        "wq": nc.dram_tensor("wq", [128, EC, D], f16, kind="ExternalInput").ap(),
        "wk": nc.dram_tensor("wk", [128, EC, D], f16, kind="ExternalInput").ap(),
        "wv": nc.dram_tensor("wv", [128, EC, D], f16, kind="ExternalInput").ap(),
        "thr": nc.dram_tensor(
            "thr", [4, NMASK, 128], f16, kind="ExternalInput").ap(),
        "ident": nc.dram_tensor(
            "ident", [128, 128], f16, kind="ExternalInput").ap(),
        "out": nc.dram_tensor("out", [4, 128, G], f32, kind="ExternalOutput").ap(),
    }
    with tile.TileContext(nc) as tc, ExitStack() as ctx:
        _emit(nc, tc, ctx, aps)
    nc.compile()
    _CACHE["nc"] = nc
    return nc


def _thresholds(h):
    """thr[s, i, k_in]: min allowed local query col for key row k_in of the
    i-th masked k-chunk (chunk c = NK[s]/128 - NMASK + i) of slot s."""
    perm = np.asarray(PERMS[h])
    thr = np.zeros((4, NMASK, 128), dtype=np.float32)
    for s in range(4):
        qc = CHUNKS[h][s]
        c0 = NK[s] // 128 - NMASK
        for i in range(NMASK):
            pos = (c0 + i) * 128 + np.arange(128)        # permuted key column
            k_orig = perm[pos // G] * G + pos % G        # original key index
            thr[s, i] = np.clip(k_orig - qc * G, 0, G)
    return thr


def make_in_maps(x, Wq, Wk, Wv):
    x = np.ascontiguousarray(x, dtype=np.float32)
    def wshape(W):
        # [E, D] -> [128, EC, D]: chunk c rows c*128..c*128+127 at [:, c, :]
        return np.ascontiguousarray(
            np.asarray(W, dtype=np.float32).reshape(EC, 128, D)
            .transpose(1, 0, 2).astype(np.float16))

    common = {
        "wq": wshape(Wq), "wk": wshape(Wk), "wv": wshape(Wv),
        "ident": np.eye(128, dtype=np.float16),
    }
    thrs = [_thresholds(0).astype(np.float16), _thresholds(1).astype(np.float16)]
    in_maps = []
    for c in range(NCORES):
        b, h = c // 2, c % 2
        xr = x[b].reshape(NG, G, E)[PERMS[h]]            # [g, t, E] permuted
        # xt[p, g, q4, i, t] = xr[g, t, (q4*4+i)*128+p]: each partition's
        # (g, q4) line is 4*512*2B = 4KB contiguous for efficient DMA.
        xt = np.ascontiguousarray(
            xr.astype(np.float16).reshape(NG, G, 4, 4, 128)
            .transpose(4, 0, 2, 3, 1))
        in_maps.append({**common, "xt": xt, "thr": thrs[h]})
    return in_maps


def gather(results):
    out = np.empty((B, T, D), dtype=np.float32)
    for c in range(NCORES):
        b, h = c // 2, c % 2
        o = results[c]["out"]                             # [4, 128, 512]
        for s in range(4):
            qc = CHUNKS[h][s]
            out[b, qc * G:(qc + 1) * G, :] = o[s].T
    return out


def run(x, Wq, Wk, Wv, trace=False, **trace_kwargs):
    from concourse.bass_utils import run_bass_kernel_spmd

    nc = _build()
    in_maps = make_in_maps(x, Wq, Wk, Wv)
    res = run_bass_kernel_spmd(
        nc, in_maps, core_ids=list(range(NCORES)), trace=trace, **trace_kwargs)
    return gather(res.results), res


def kernel(x, Wq, Wk, Wv):
    out, _ = run(np.asarray(x), np.asarray(Wq), np.asarray(Wk), np.asarray(Wv))
    return out
